# revision 15
# baseline (speedup 1.0000x reference)
"""Trainium2 Bass kernel for EfficientAttention (linear attention block).

Computation (per batch b, head h):
    qkv = x @ w_qkv.T (+ b_qkv)
    q = softmax(q, axis=head_dim) * head_dim**-0.5
    k = softmax(k, axis=seqlen)
    kv[d,e] = sum_s k[s,d] v[s,e]          (per-head 64x64 state)
    out[s,e] = sum_d q[s,d] kv[d,e]
    y = out @ w_proj.T (+ b_proj)

Sharding: 8 cores = (batch b = c//2, seq half = c%2); 2048 tokens per core,
all 16 heads. Cross-core coupling: the kv state and the k-softmax
denominator Z (sums over the full 4096 seqlen) -> one [128, 520] bf16
AllReduce across seq-half pairs.

Fast-path layout (no-bias variant, which is what zero-bias inputs use):
- q/k projections run in fp8(e4m3) DoubleRow mode (2 contraction chunks
  per instruction, 2x PE rate); inputs are host-scaled (x*16, w*512) to
  dodge fp8 subnormals and the exp() activation descales by 1/8192.
  v/out projections stay bf16 (their error hits the output linearly).
- All other operands bf16 on-chip; PSUM accumulation fp32.
- x resident in SBUF as per-token-block tiles (tile-granular dependency
  tracking makes a single big tile serialize on all its chunk DMAs).
- Z is fused into the kv-state matmul via a ones column appended to
  each v pair block.
- Program order: k/v sweep -> collective trigger -> whole q sweep
  (hides the AllReduce) -> attention + output projection interleaved
  per 512-token chunk (separate per-chunk q tiles avoid WAR stalls).
- DMA priority: wk + x block 0 first on the sync queue; wq/wp loads are
  gated behind tb0 so they don't steal startup HBM bandwidth.
"""

import sys

sys.path.insert(0, "/opt/trn_rl_repo")

import numpy as np
import ml_dtypes

import concourse.bacc as bacc
import concourse.tile as tile
from concourse import mybir
from concourse import bass_utils

F32 = mybir.dt.float32
BF16 = mybir.dt.bfloat16
FP8 = mybir.dt.float8e4

D = 1024          # model dim (= qkv contraction dim)
T = 2048          # tokens per core (one batch element's half sequence)
NH = 16           # heads
HD = 64           # head dim
NPAIR = 8         # head pairs (2 heads / 128 partitions)
KC = D // 128     # contraction chunks of 128
TB = T // 128     # token blocks of 128
SCALE = HD ** -0.5

SX = 16.0         # host scale on fp8 x
SW = 512.0        # host scale on fp8 wq/wk
DESCALE = 1.0 / (SX * SW)

N_CORES = 8
FP8_QK = True     # fp8 DoubleRow q/k projections in the no-bias program


def bias_bcast(b):
    # DRAM [D] broadcast-load to SBUF [128, D] (partition step 0)
    import concourse.bass as bass
    ap = b[:]
    return bass.AP(tensor=ap.tensor, offset=ap.offset, ap=[[0, 128]] + list(ap.ap))


def build_program(with_bias=False):
    fp8_qk = FP8_QK and not with_bias
    nc = bacc.Bacc("TRN2", target_bir_lowering=False, num_devices=N_CORES)

    xt = nc.dram_tensor("xt", [D, T], BF16, kind="ExternalInput")      # x.T
    wv = nc.dram_tensor("wv", [D, D], BF16, kind="ExternalInput")      # w_v.T
    wp = nc.dram_tensor("wp", [D, D], BF16, kind="ExternalInput")      # w_proj.T
    if fp8_qk:
        xf = nc.dram_tensor("xf", [D, T], FP8, kind="ExternalInput")   # x.T * SX
        wqf = nc.dram_tensor("wqf", [D, D], FP8, kind="ExternalInput")  # w_q.T * SW
        wkf = nc.dram_tensor("wkf", [D, D], FP8, kind="ExternalInput")  # w_k.T * SW
    else:
        wq = nc.dram_tensor("wq", [D, D], BF16, kind="ExternalInput")
        wk = nc.dram_tensor("wk", [D, D], BF16, kind="ExternalInput")
    bq = nc.dram_tensor("bq", [D], F32, kind="ExternalInput")
    bk = nc.dram_tensor("bk", [D], F32, kind="ExternalInput")
    bv = nc.dram_tensor("bv", [D], F32, kind="ExternalInput")
    bp = nc.dram_tensor("bp", [D], F32, kind="ExternalInput")
    cst = nc.dram_tensor("cst", [128, 136], BF16, kind="ExternalInput")  # identity | ones8
    y = nc.dram_tensor("y", [T, D], F32, kind="ExternalOutput")

    xt_v = xt.rearrange("(kc p) t -> p kc t", p=128)
    wv_v = wv.rearrange("(kc p) f -> p kc f", p=128)
    wp_v = wp.rearrange("(kc p) f -> p kc f", p=128)
    if fp8_qk:
        xf_v = xf.rearrange("(kc p) t -> p kc t", p=128)
        wqf_v = wqf.rearrange("(kc p) f -> p kc f", p=128)
        wkf_v = wkf.rearrange("(kc p) f -> p kc f", p=128)
    else:
        wq_v = wq.rearrange("(kc p) f -> p kc f", p=128)
        wk_v = wk.rearrange("(kc p) f -> p kc f", p=128)

    TCN = T // 512
    DR = mybir.MatmulPerfMode.DoubleRow

    with tile.TileContext(nc) as tc:
        with (
            tc.tile_pool(name="const", bufs=1) as const,
            tc.tile_pool(name="wpool", bufs=1) as wpool,
            tc.tile_pool(name="xp", bufs=1) as xp,
            tc.tile_pool(name="ekv", bufs=2) as ekv,
            tc.tile_pool(name="acc", bufs=1) as accp,
            tc.tile_pool(name="qpool", bufs=2) as qpool,
            tc.tile_pool(name="qt", bufs=1) as qtpool,
            tc.tile_pool(name="kvsb", bufs=1) as kvsbp,
            tc.tile_pool(name="yout", bufs=3) as youtp,
            tc.tile_pool(name="psum", bufs=3, space="PSUM") as psum,
            tc.tile_pool(name="dram", bufs=1, space="DRAM") as dram,
        ):
            cst_sb = const.tile([128, 136], BF16, tag="cst")
            nc.sync.dma_start(cst_sb, cst[:])
            ident = cst_sb[:, 0:128]

            # ---- weight tiles ----
            wv_sb = [wpool.tile([128, D], BF16, tag=f"wv{kc}", name=f"wv{kc}")
                     for kc in range(KC)]
            wp_sb = [wpool.tile([128, D], BF16, tag=f"wp{kc}", name=f"wp{kc}")
                     for kc in range(KC)]
            if fp8_qk:
                wkf_sb = wpool.tile([128, KC, D], FP8, tag="wkf", name="wkf")
                wqf_sb = wpool.tile([128, KC, D], FP8, tag="wqf", name="wqf")
                # sub-chunked so the pieces spread across DMA engines and
                # complete early (startup critical path)
                for kc in range(KC):
                    for hh in range(2):
                        nc.sync.dma_start(
                            wkf_sb[:, kc, 512 * hh:512 * (hh + 1)],
                            wkf_v[:, kc, 512 * hh:512 * (hh + 1)])
            else:
                wk_sb = [wpool.tile([128, D], BF16, tag=f"wk{kc}", name=f"wk{kc}")
                         for kc in range(KC)]
                wq_sb = [wpool.tile([128, D], BF16, tag=f"wq{kc}", name=f"wq{kc}")
                         for kc in range(KC)]
                for kc in range(KC):
                    nc.sync.dma_start(wk_sb[kc], wk_v[:, kc, :])

            if with_bias:
                bk_sb = const.tile([128, D], BF16, tag="bk")
                bv_sb = const.tile([128, D], BF16, tag="bv")
                bq_sb = const.tile([128, D], BF16, tag="bq")
                bp_sb = const.tile([128, D], BF16, tag="bp")
                nc.gpsimd.dma_start(bk_sb, bias_bcast(bk))
                nc.gpsimd.dma_start(bv_sb, bias_bcast(bv))
                nc.gpsimd.dma_start(bq_sb, bias_bcast(bq))
                nc.gpsimd.dma_start(bp_sb, bias_bcast(bp))

            # x resident in SBUF as per-tb tiles (tile-granular dep tracking:
            # one big tile would make tb0 wait for every chunk DMA)
            xts = [xp.tile([128, KC, 128], BF16, tag=f"xt{tb}", name=f"xt{tb}")
                   for tb in range(TB)]
            if fp8_qk:
                xfs = [xp.tile([128, KC, 128], FP8, tag=f"xf{tb}", name=f"xf{tb}")
                       for tb in range(TB)]

            def load_x(tb):
                if fp8_qk:
                    nc.sync.dma_start(xfs[tb], xf_v[:, :, tb * 128:(tb + 1) * 128])
                nc.sync.dma_start(xts[tb], xt_v[:, :, tb * 128:(tb + 1) * 128])

            load_x(0)
            # second gate: hold the rest of the sync-queue loads until the
            # startup-critical ones (wk, x block 0) have actually landed
            gate2 = dram.tile([1, 8], FP8 if fp8_qk else BF16, tag="gate2")
            if fp8_qk:
                nc.sync.dma_start(gate2, xfs[0][0:1, 0, 0:8])
            else:
                nc.sync.dma_start(gate2, xts[0][0:1, 0, 0:8])
            for kc in range(KC):
                nc.sync.dma_start(wv_sb[kc], wv_v[:, kc, :])
            for tb in range(1, TB):
                load_x(tb)

            # v tiles with a ones column per pair block: [v_pair(128) | 1] x 8
            vv_t = [accp.tile([128, NPAIR * 129], BF16, tag=f"v{i}", name=f"vv{i}")
                    for i in range(2)]
            for i in range(2):
                nc.vector.tensor_copy(
                    vv_t[i][:].rearrange("q (p c) -> q p c", c=129)[:, :, 128],
                    cst_sb[:, 128:136])

            def proj_qk(ps, tb, w_sb, sl):
                # q/k projection into psum: fp8 DoubleRow (2 kc per mm) or bf16
                if fp8_qk:
                    for g in range(KC // 2):
                        nc.tensor.matmul(ps, xfs[tb][:, 2 * g:2 * g + 2, :],
                                         w_sb[:, 2 * g:2 * g + 2, sl],
                                         start=(g == 0), stop=(g == KC // 2 - 1),
                                         perf_mode=DR)
                else:
                    for kc in range(KC):
                        nc.tensor.matmul(ps, xts[tb][:, kc, :], w_sb[kc][:, sl],
                                         start=(kc == 0), stop=(kc == KC - 1))

            exp_scale = DESCALE if fp8_qk else 1.0

            # ---- Phase B: k/v projections, exp(k), partial [kv | Z] ----
            kvacc = accp.tile([128, NPAIR * 129], F32, tag="kvacc")

            def phase_b_tb(tb):
                ek = ekv.tile([128, D], BF16, tag="ek", name="ek")
                vv = vv_t[tb % 2]
                for half in range(2):
                    sl = slice(half * 512, (half + 1) * 512)
                    ps = psum.tile([128, 512], F32, tag="mm")
                    proj_qk(ps, tb, wkf_sb if fp8_qk else wk_sb, sl)
                    if with_bias:
                        nc.vector.tensor_add(ps, ps, bk_sb[:, sl])
                    nc.scalar.activation(ek[:, sl], ps,
                                         mybir.ActivationFunctionType.Exp,
                                         scale=exp_scale)
                    ps = psum.tile([128, 512], F32, tag="mm")
                    for kc in range(KC):
                        nc.tensor.matmul(ps, xts[tb][:, kc, :], wv_sb[kc][:, sl],
                                         start=(kc == 0), stop=(kc == KC - 1))
                    if with_bias:
                        nc.vector.tensor_add(ps, ps, bv_sb[:, sl])
                    for j in range(4):
                        p = 4 * half + j
                        nc.scalar.copy(vv[:, 129 * p:129 * p + 128],
                                       ps[:, 128 * j:128 * j + 128])
                # [kv | Z] partials: per pair, lhsT = ek pair cols,
                # rhs = [v pair cols | ones] (129 moving)
                for g in range(4):
                    kps = psum.tile([128, 258], F32, tag="kv", bufs=2)
                    for j in range(2):
                        p = 2 * g + j
                        nc.tensor.matmul(
                            kps[:, 129 * j:129 * j + 129],
                            ek[:, 128 * p:128 * p + 128],
                            vv[:, 129 * p:129 * p + 129],
                            start=True, stop=True)
                    if tb == 0:
                        nc.vector.tensor_copy(kvacc[:, 258 * g:258 * (g + 1)], kps)
                    else:
                        nc.vector.tensor_add(kvacc[:, 258 * g:258 * (g + 1)],
                                             kvacc[:, 258 * g:258 * (g + 1)], kps)
                return ek

            ek0 = phase_b_tb(0)
            # release wq/wp weight loads only once tb0 is in flight: a tiny
            # gpsimd DMA depending on ek0 stalls the gpsimd queue (in-order)
            # so the big loads don't compete with wk/wv/x for HBM at startup
            gate = dram.tile([1, 8], BF16, tag="gate")
            nc.gpsimd.dma_start(gate, ek0[0:1, 0:8])
            if fp8_qk:
                for kc in range(KC):
                    nc.gpsimd.dma_start(wqf_sb[:, kc, :], wqf_v[:, kc, :])
            else:
                for kc in range(KC):
                    nc.gpsimd.dma_start(wq_sb[kc], wq_v[:, kc, :])
            for kc in range(KC):
                nc.gpsimd.dma_start(wp_sb[kc], wp_v[:, kc, :])
            for tb in range(1, TB):
                phase_b_tb(tb)

            # ---- stage = whole [kv | Z] pair-block accumulator cast to bf16
            # in ONE op (keeps the DVE queue clear for the q sweep); the
            # AllReduce carries the cross-head garbage blocks too (264 KB),
            # which stays hidden under the q sweep
            stage = accp.tile([128, NPAIR * 129], BF16, tag="stage")
            nc.vector.tensor_copy(stage, kvacc)
            cin = dram.tile([128, NPAIR * 129], BF16, tag="cin")
            cout = dram.tile([128, NPAIR * 129], BF16, tag="cout")
            nc.sync.dma_start(cin, stage)
            nc.gpsimd.collective_compute(
                "AllReduce", mybir.AluOpType.add,
                replica_groups=[[0, 1], [2, 3], [4, 5], [6, 7]],
                ins=[cin[:].opt()], outs=[cout[:].opt()])

            # ---- q sweep (independent of the collective; hides it) ----
            # per-512-token-chunk q tiles so later attn writes don't WAR-stall
            qt4 = [qtpool.tile([128, NPAIR, 512], BF16, tag=f"qt{i}", name=f"qt{i}")
                   for i in range(TCN)]
            import concourse.bass as bass
            for tb in range(TB):
                eq = qpool.tile([128, D], BF16, tag="eq")
                sums = qpool.tile([128, NH], F32, tag="sums")
                for half in range(2):
                    sl = slice(half * 512, (half + 1) * 512)
                    ps = psum.tile([128, 512], F32, tag="mm")
                    proj_qk(ps, tb, wqf_sb if fp8_qk else wq_sb, sl)
                    if with_bias:
                        nc.vector.tensor_add(ps, ps, bq_sb[:, sl])
                    # per-head exp with accumulated row sums (no DVE reduce)
                    for hh in range(8):
                        h = 8 * half + hh
                        nc.scalar.activation(
                            eq[:, h * HD:(h + 1) * HD],
                            ps[:, hh * HD:(hh + 1) * HD],
                            mybir.ActivationFunctionType.Exp,
                            scale=exp_scale,
                            accum_out=sums[:, h:h + 1])
                rfac = qpool.tile([128, NH], F32, tag="rfac")
                nc.vector.reciprocal(rfac, sums)
                nc.scalar.mul(rfac, rfac, SCALE)
                # normalize all 16 heads in one DVE op: rfac broadcast along
                # head_dim via a stride-0 inner AP
                rap = rfac[:, :]
                rbc = bass.AP(tensor=rap.tensor, offset=rap.offset,
                              ap=[list(rap.ap[0]), list(rap.ap[1]), [0, HD]])
                eqv = eq[:].rearrange("p (h e) -> p h e", e=HD)
                nc.vector.tensor_mul(eqv, eqv, rbc)
                qdst = qt4[tb // 4]
                toff = (tb % 4) * 128
                for g4 in range(2):
                    tp = psum.tile([128, 512], BF16, tag="tr", bufs=2)
                    for j in range(4):
                        p = 4 * g4 + j
                        nc.tensor.transpose(tp[:, j * 128:(j + 1) * 128],
                                            eq[:, p * 128:(p + 1) * 128], ident)
                    dst = qdst[:, 4 * g4:4 * g4 + 4, toff:toff + 128]
                    src = tp[:].rearrange("p (j t) -> p j t", j=4)
                    if g4 == 0:
                        nc.vector.tensor_copy(dst, src)
                    else:
                        nc.scalar.copy(dst, src)

            # ---- collective result -> normalized per-pair kv blocks ----
            kvred = accp.tile([128, NPAIR * 129], BF16, tag="kvred")
            nc.sync.dma_start(kvred, cout)
            rz = accp.tile([128, NPAIR], F32, tag="rz")
            nc.vector.reciprocal(
                rz, kvred[:].rearrange("q (p c) -> q p c", c=129)[:, :, 128])
            kv_sb = [kvsbp.tile([128, 128], BF16, tag=f"kv{p}", name=f"kv{p}")
                     for p in range(NPAIR)]
            for p in range(NPAIR):
                c0 = 129 * p
                # off-diagonal head-cross blocks must be exact zeros
                nc.vector.tensor_scalar_mul(
                    kv_sb[p][0:64, 64:128], kvred[0:64, c0:c0 + 64], 0.0)
                nc.vector.tensor_scalar_mul(
                    kv_sb[p][64:128, 0:64], kvred[64:128, c0:c0 + 64], 0.0)
                nc.vector.tensor_scalar_mul(
                    kv_sb[p][0:64, 0:64], kvred[0:64, c0:c0 + 64],
                    rz[0:64, p:p + 1])
                nc.vector.tensor_scalar_mul(
                    kv_sb[p][64:128, 64:128], kvred[64:128, c0 + 64:c0 + 128],
                    rz[64:128, p:p + 1])

            # ---- attention out + output projection, interleaved per
            # 512-token chunk; attn result overwrites the chunk's q tile
            for tcn in range(TCN):
                qtc = qt4[tcn]
                for p in range(NPAIR):
                    aps = psum.tile([128, 512], F32, tag="mm")
                    nc.tensor.matmul(aps, kv_sb[p], qtc[:, p, :],
                                     start=True, stop=True)
                    # split copybacks across ACT/DVE so neither engine gates
                    if p % 2 == 0:
                        nc.scalar.copy(qtc[:, p, :], aps)
                    else:
                        nc.vector.tensor_copy(qtc[:, p, :], aps)
                for t4 in range(4):
                    tb = 4 * tcn + t4
                    for oc in range(2):
                        osl = slice(oc * 512, (oc + 1) * 512)
                        ps = psum.tile([128, 512], F32, tag="mm")
                        for kc in range(KC):
                            nc.tensor.matmul(
                                ps, qtc[:, kc, t4 * 128:(t4 + 1) * 128],
                                wp_sb[kc][:, osl],
                                start=(kc == 0), stop=(kc == KC - 1))
                        yt = youtp.tile([128, 512], F32, tag="y")
                        if with_bias:
                            nc.vector.tensor_add(yt, ps, bp_sb[:, osl])
                        elif oc == 0:
                            nc.vector.tensor_copy(yt, ps)
                        else:
                            nc.scalar.copy(yt, ps)
                        nc.sync.dma_start(y[tb * 128:(tb + 1) * 128, osl], yt)

    nc.compile()
    return nc


_NC = {}


def _get_nc(with_bias=False):
    if with_bias not in _NC:
        _NC[with_bias] = build_program(with_bias=with_bias)
    return _NC[with_bias]


def kernel(x, w_qkv, b_qkv, w_proj, b_proj):
    x = np.asarray(x, dtype=np.float32)
    w_qkv = np.asarray(w_qkv, dtype=np.float32)
    b_qkv = np.asarray(b_qkv, dtype=np.float32)
    w_proj = np.asarray(w_proj, dtype=np.float32)
    b_proj = np.asarray(b_proj, dtype=np.float32)

    bs, seqlen, dim = x.shape
    half = seqlen // 2

    bf16 = ml_dtypes.bfloat16
    fp8 = ml_dtypes.float8_e4m3
    with_bias = bool(np.any(b_qkv)) or bool(np.any(b_proj))
    fp8_qk = FP8_QK and not with_bias

    wqT = np.ascontiguousarray(w_qkv[0:D].T)
    wkT = np.ascontiguousarray(w_qkv[D:2 * D].T)
    wvT = np.ascontiguousarray(w_qkv[2 * D:3 * D].T).astype(bf16)
    wpT = np.ascontiguousarray(w_proj.T).astype(bf16)
    bq, bk, bv = b_qkv[0:D], b_qkv[D:2 * D], b_qkv[2 * D:3 * D]

    cst = np.concatenate(
        [np.eye(128, dtype=np.float32),
         np.ones((128, 8), dtype=np.float32)], axis=1).astype(bf16)

    common = {
        "wv": wvT, "wp": wpT, "cst": cst,
        "bq": np.ascontiguousarray(bq), "bk": np.ascontiguousarray(bk),
        "bv": np.ascontiguousarray(bv), "bp": np.ascontiguousarray(b_proj),
    }
    if fp8_qk:
        common["wqf"] = (wqT * SW).astype(fp8)
        common["wkf"] = (wkT * SW).astype(fp8)
    else:
        common["wq"] = wqT.astype(bf16)
        common["wk"] = wkT.astype(bf16)

    in_maps = []
    for c in range(N_CORES):
        b, s = divmod(c, 2)
        chunk = np.ascontiguousarray(x[b, s * half:(s + 1) * half, :].T)
        m = dict(common)
        m["xt"] = chunk.astype(bf16)
        if fp8_qk:
            m["xf"] = (chunk * SX).astype(fp8)
        in_maps.append(m)

    nc = _get_nc(with_bias)
    global _last_in_maps, _last_with_bias
    _last_in_maps = in_maps
    _last_with_bias = with_bias
    res = bass_utils.run_bass_kernel_spmd(nc, in_maps, core_ids=list(range(N_CORES)))

    out = np.empty((bs, seqlen, dim), dtype=np.float32)
    for c in range(N_CORES):
        b, s = divmod(c, 2)
        out[b, s * half:(s + 1) * half, :] = res.results[c]["y"]
    return out


# revision 19
# speedup vs baseline: 1.2484x; 1.2484x over previous
"""Trainium2 Bass kernel for EfficientAttention (linear attention block).

Computation (per batch b, head h):
    qkv = x @ w_qkv.T (+ b_qkv)
    q = softmax(q, axis=head_dim) * head_dim**-0.5
    k = softmax(k, axis=seqlen)
    kv[d,e] = sum_s k[s,d] v[s,e]          (per-head 64x64 state)
    out[s,e] = sum_d q[s,d] kv[d,e]
    y = out @ w_proj.T (+ b_proj)

Sharding: 8 cores = (batch b = c//2, seq half = c%2); 2048 tokens per core,
all 16 heads. Cross-core coupling: the kv state and the k-softmax
denominator Z (sums over the full 4096 seqlen) -> one [128, 520] bf16
AllReduce across seq-half pairs.

Fast-path layout (no-bias variant, which is what zero-bias inputs use):
- q/k projections run in fp8(e4m3) DoubleRow mode (2 contraction chunks
  per instruction, 2x PE rate); inputs are host-scaled (x*16, w*512) to
  dodge fp8 subnormals and the exp() activation descales by 1/8192.
  v/out projections stay bf16 (their error hits the output linearly).
- All other operands bf16 on-chip; PSUM accumulation fp32.
- x resident in SBUF as per-token-block tiles (tile-granular dependency
  tracking makes a single big tile serialize on all its chunk DMAs).
- Z is fused into the kv-state matmul via a ones column appended to
  each v pair block.
- Program order: k/v sweep -> collective trigger -> whole q sweep
  (hides the AllReduce) -> attention + output projection interleaved
  per 512-token chunk (separate per-chunk q tiles avoid WAR stalls).
- DMA priority: wk + x block 0 first on the sync queue; wq/wp loads are
  gated behind tb0 so they don't steal startup HBM bandwidth.
"""

import sys

sys.path.insert(0, "/opt/trn_rl_repo")

import numpy as np
import ml_dtypes

import concourse.bacc as bacc
import concourse.tile as tile
from concourse import mybir
from concourse import bass_utils

F32 = mybir.dt.float32
BF16 = mybir.dt.bfloat16
FP8 = mybir.dt.float8e4

D = 1024          # model dim (= qkv contraction dim)
T = 2048          # tokens per core (one batch element's half sequence)
NH = 16           # heads
HD = 64           # head dim
NPAIR = 8         # head pairs (2 heads / 128 partitions)
KC = D // 128     # contraction chunks of 128
TB = T // 128     # token blocks of 128
SCALE = HD ** -0.5

SX = 16.0         # host scale on fp8 x
SW = 512.0        # host scale on fp8 wq/wk
DESCALE = 1.0 / (SX * SW)

N_CORES = 8
FP8_QK = True     # fp8 DoubleRow q/k projections in the no-bias program


def bias_bcast(b):
    # DRAM [D] broadcast-load to SBUF [128, D] (partition step 0)
    import concourse.bass as bass
    ap = b[:]
    return bass.AP(tensor=ap.tensor, offset=ap.offset, ap=[[0, 128]] + list(ap.ap))


def build_program(with_bias=False):
    fp8_qk = FP8_QK and not with_bias
    nc = bacc.Bacc("TRN2", target_bir_lowering=False, num_devices=N_CORES)

    xt = nc.dram_tensor("xt", [D, T], BF16, kind="ExternalInput")      # x.T
    wv = nc.dram_tensor("wv", [D, D], BF16, kind="ExternalInput")      # w_v.T
    wp = nc.dram_tensor("wp", [D, D], BF16, kind="ExternalInput")      # w_proj.T
    if fp8_qk:
        xf = nc.dram_tensor("xf", [D, T], FP8, kind="ExternalInput")   # x.T * SX
        wqf = nc.dram_tensor("wqf", [D, D], FP8, kind="ExternalInput")  # w_q.T * SW
        wkf = nc.dram_tensor("wkf", [D, D], FP8, kind="ExternalInput")  # w_k.T * SW
    else:
        wq = nc.dram_tensor("wq", [D, D], BF16, kind="ExternalInput")
        wk = nc.dram_tensor("wk", [D, D], BF16, kind="ExternalInput")
    bq = nc.dram_tensor("bq", [D], F32, kind="ExternalInput")
    bk = nc.dram_tensor("bk", [D], F32, kind="ExternalInput")
    bv = nc.dram_tensor("bv", [D], F32, kind="ExternalInput")
    bp = nc.dram_tensor("bp", [D], F32, kind="ExternalInput")
    cst = nc.dram_tensor("cst", [128, 136], BF16, kind="ExternalInput")  # identity | ones8
    y = nc.dram_tensor("y", [T, D], F32, kind="ExternalOutput")

    xt_v = xt.rearrange("(kc p) t -> p kc t", p=128)
    wv_v = wv.rearrange("(kc p) f -> p kc f", p=128)
    wp_v = wp.rearrange("(kc p) f -> p kc f", p=128)
    if fp8_qk:
        xf_v = xf.rearrange("(kc p) t -> p kc t", p=128)
        wqf_v = wqf.rearrange("(kc p) f -> p kc f", p=128)
        wkf_v = wkf.rearrange("(kc p) f -> p kc f", p=128)
    else:
        wq_v = wq.rearrange("(kc p) f -> p kc f", p=128)
        wk_v = wk.rearrange("(kc p) f -> p kc f", p=128)

    TCN = T // 512
    DR = mybir.MatmulPerfMode.DoubleRow

    with tile.TileContext(nc) as tc:
        with (
            tc.tile_pool(name="const", bufs=1) as const,
            tc.tile_pool(name="wpool", bufs=1) as wpool,
            tc.tile_pool(name="xp", bufs=1) as xp,
            tc.tile_pool(name="ekv", bufs=2) as ekv,
            tc.tile_pool(name="acc", bufs=1) as accp,
            tc.tile_pool(name="qpool", bufs=2) as qpool,
            tc.tile_pool(name="qt", bufs=1) as qtpool,
            tc.tile_pool(name="kvsb", bufs=1) as kvsbp,
            tc.tile_pool(name="yout", bufs=3) as youtp,
            tc.tile_pool(name="psum", bufs=3, space="PSUM") as psum,
            tc.tile_pool(name="dram", bufs=1, space="DRAM") as dram,
        ):
            cst_sb = const.tile([128, 136], BF16, tag="cst")
            nc.sync.dma_start(cst_sb, cst[:])
            ident = cst_sb[:, 0:128]

            # ---- weight tiles ----
            wv_sb = [wpool.tile([128, D], BF16, tag=f"wv{kc}", name=f"wv{kc}")
                     for kc in range(KC)]
            wp_sb = [wpool.tile([128, D], BF16, tag=f"wp{kc}", name=f"wp{kc}")
                     for kc in range(KC)]
            if fp8_qk:
                wkf_sb = wpool.tile([128, KC, D], FP8, tag="wkf", name="wkf")
                wqf_sb = wpool.tile([128, KC, D], FP8, tag="wqf", name="wqf")
                # sub-chunked so the pieces spread across DMA engines and
                # complete early (startup critical path)
                for kc in range(KC):
                    for hh in range(2):
                        nc.sync.dma_start(
                            wkf_sb[:, kc, 512 * hh:512 * (hh + 1)],
                            wkf_v[:, kc, 512 * hh:512 * (hh + 1)])
            else:
                wk_sb = [wpool.tile([128, D], BF16, tag=f"wk{kc}", name=f"wk{kc}")
                         for kc in range(KC)]
                wq_sb = [wpool.tile([128, D], BF16, tag=f"wq{kc}", name=f"wq{kc}")
                         for kc in range(KC)]
                for kc in range(KC):
                    nc.sync.dma_start(wk_sb[kc], wk_v[:, kc, :])

            if with_bias:
                bk_sb = const.tile([128, D], BF16, tag="bk")
                bv_sb = const.tile([128, D], BF16, tag="bv")
                bq_sb = const.tile([128, D], BF16, tag="bq")
                bp_sb = const.tile([128, D], BF16, tag="bp")
                nc.gpsimd.dma_start(bk_sb, bias_bcast(bk))
                nc.gpsimd.dma_start(bv_sb, bias_bcast(bv))
                nc.gpsimd.dma_start(bq_sb, bias_bcast(bq))
                nc.gpsimd.dma_start(bp_sb, bias_bcast(bp))

            # x resident in SBUF as per-tb tiles (tile-granular dep tracking:
            # one big tile would make tb0 wait for every chunk DMA)
            xts = [xp.tile([128, KC, 128], BF16, tag=f"xt{tb}", name=f"xt{tb}")
                   for tb in range(TB)]
            if fp8_qk:
                xfs = [xp.tile([128, KC, 128], FP8, tag=f"xf{tb}", name=f"xf{tb}")
                       for tb in range(TB)]

            def load_x(tb, eng):
                if fp8_qk:
                    eng.dma_start(xfs[tb], xf_v[:, :, tb * 128:(tb + 1) * 128])
                eng.dma_start(xts[tb], xt_v[:, :, tb * 128:(tb + 1) * 128])

            load_x(0, nc.sync)
            for kc in range(KC):
                nc.sync.dma_start(wv_sb[kc], wv_v[:, kc, :])
            for tb in range(1, TB):
                load_x(tb, nc.sync)

            # v tiles with a ones column per pair block: [v_pair(128) | 1] x 8
            vv_t = [accp.tile([128, NPAIR * 129], BF16, tag=f"v{i}", name=f"vv{i}")
                    for i in range(2)]
            for i in range(2):
                nc.vector.tensor_copy(
                    vv_t[i][:].rearrange("q (p c) -> q p c", c=129)[:, :, 128],
                    cst_sb[:, 128:136])

            def proj_qk(ps, tb, w_sb, sl):
                # q/k projection into psum: fp8 DoubleRow (2 kc per mm) or bf16
                if fp8_qk:
                    for g in range(KC // 2):
                        nc.tensor.matmul(ps, xfs[tb][:, 2 * g:2 * g + 2, :],
                                         w_sb[:, 2 * g:2 * g + 2, sl],
                                         start=(g == 0), stop=(g == KC // 2 - 1),
                                         perf_mode=DR)
                else:
                    for kc in range(KC):
                        nc.tensor.matmul(ps, xts[tb][:, kc, :], w_sb[kc][:, sl],
                                         start=(kc == 0), stop=(kc == KC - 1))

            exp_scale = DESCALE if fp8_qk else 1.0

            # ---- Phase B: k/v projections, exp(k), partial [kv | Z] ----
            kvacc = accp.tile([128, NPAIR * 129], F32, tag="kvacc")

            def phase_b_tb(tb):
                ek = ekv.tile([128, D], BF16, tag="ek", name="ek")
                vv = vv_t[tb % 2]
                for half in range(2):
                    sl = slice(half * 512, (half + 1) * 512)
                    ps = psum.tile([128, 512], F32, tag="mm")
                    proj_qk(ps, tb, wkf_sb if fp8_qk else wk_sb, sl)
                    if with_bias:
                        nc.vector.tensor_add(ps, ps, bk_sb[:, sl])
                    nc.scalar.activation(ek[:, sl], ps,
                                         mybir.ActivationFunctionType.Exp,
                                         scale=exp_scale)
                    ps = psum.tile([128, 512], F32, tag="mm")
                    for kc in range(KC):
                        nc.tensor.matmul(ps, xts[tb][:, kc, :], wv_sb[kc][:, sl],
                                         start=(kc == 0), stop=(kc == KC - 1))
                    if with_bias:
                        nc.vector.tensor_add(ps, ps, bv_sb[:, sl])
                    for j in range(4):
                        p = 4 * half + j
                        dst = vv[:, 129 * p:129 * p + 128]
                        src = ps[:, 128 * j:128 * j + 128]
                        if j % 2 == 0:
                            nc.scalar.copy(dst, src)
                        else:
                            nc.vector.tensor_copy(dst, src)
                # [kv | Z] partials: per pair, lhsT = ek pair cols,
                # rhs = [v pair cols | ones] (129 moving)
                for g in range(4):
                    kps = psum.tile([128, 258], F32, tag="kv", bufs=2)
                    for j in range(2):
                        p = 2 * g + j
                        nc.tensor.matmul(
                            kps[:, 129 * j:129 * j + 129],
                            ek[:, 128 * p:128 * p + 128],
                            vv[:, 129 * p:129 * p + 129],
                            start=True, stop=True)
                    if tb == 0:
                        nc.vector.tensor_copy(kvacc[:, 258 * g:258 * (g + 1)], kps)
                    else:
                        nc.vector.tensor_add(kvacc[:, 258 * g:258 * (g + 1)],
                                             kvacc[:, 258 * g:258 * (g + 1)], kps)
                return ek

            ek0 = phase_b_tb(0)
            # release wq/wp weight loads only once tb0 is in flight: a tiny
            # gpsimd DMA depending on ek0 stalls the gpsimd queue (in-order)
            # so the big loads don't compete with wk/wv/x for HBM at startup
            gate = dram.tile([1, 8], BF16, tag="gate")
            nc.gpsimd.dma_start(gate, ek0[0:1, 0:8])
            if fp8_qk:
                for kc in range(KC):
                    nc.gpsimd.dma_start(wqf_sb[:, kc, :], wqf_v[:, kc, :])
            else:
                for kc in range(KC):
                    nc.gpsimd.dma_start(wq_sb[kc], wq_v[:, kc, :])
            for kc in range(KC):
                nc.gpsimd.dma_start(wp_sb[kc], wp_v[:, kc, :])
            for tb in range(1, TB):
                phase_b_tb(tb)

            # ---- stage = whole [kv | Z] pair-block accumulator cast to bf16
            # in ONE op (keeps the DVE queue clear for the q sweep); the
            # AllReduce carries the cross-head garbage blocks too (264 KB),
            # which stays hidden under the q sweep
            stage = accp.tile([128, NPAIR * 129], BF16, tag="stage")
            nc.vector.tensor_copy(stage, kvacc)
            cin = dram.tile([128, NPAIR * 129], BF16, tag="cin")
            cout = dram.tile([128, NPAIR * 129], BF16, tag="cout")
            nc.sync.dma_start(cin, stage)
            nc.gpsimd.collective_compute(
                "AllReduce", mybir.AluOpType.add,
                replica_groups=[[0, 1], [2, 3], [4, 5], [6, 7]],
                ins=[cin[:].opt()], outs=[cout[:].opt()])

            # ---- q sweep (independent of the collective; hides it) ----
            # per-512-token-chunk q tiles so later attn writes don't WAR-stall
            qt4 = [qtpool.tile([128, NPAIR, 512], BF16, tag=f"qt{i}", name=f"qt{i}")
                   for i in range(TCN)]
            import concourse.bass as bass
            for tb in range(TB):
                eq = qpool.tile([128, D], BF16, tag="eq")
                sums = qpool.tile([128, NH], F32, tag="sums")
                for half in range(2):
                    sl = slice(half * 512, (half + 1) * 512)
                    ps = psum.tile([128, 512], F32, tag="mm")
                    proj_qk(ps, tb, wqf_sb if fp8_qk else wq_sb, sl)
                    if with_bias:
                        nc.vector.tensor_add(ps, ps, bq_sb[:, sl])
                    nc.scalar.activation(eq[:, sl], ps,
                                         mybir.ActivationFunctionType.Exp,
                                         scale=exp_scale)
                    # half-granular row sums pipeline behind the exps
                    nc.vector.reduce_sum(
                        sums[:, 8 * half:8 * half + 8],
                        eq[:, sl].rearrange("p (h e) -> p h e", e=HD),
                        axis=mybir.AxisListType.X)
                rfac = qpool.tile([128, NH], F32, tag="rfac")
                nc.vector.reciprocal(rfac, sums)
                nc.scalar.mul(rfac, rfac, SCALE)
                # normalize all 16 heads in one DVE op: rfac broadcast along
                # head_dim via a stride-0 inner AP
                rap = rfac[:, :]
                rbc = bass.AP(tensor=rap.tensor, offset=rap.offset,
                              ap=[list(rap.ap[0]), list(rap.ap[1]), [0, HD]])
                eqv = eq[:].rearrange("p (h e) -> p h e", e=HD)
                nc.vector.tensor_mul(eqv, eqv, rbc)
                qdst = qt4[tb // 4]
                toff = (tb % 4) * 128
                for g4 in range(2):
                    tp = psum.tile([128, 512], BF16, tag="tr", bufs=2)
                    for j in range(4):
                        p = 4 * g4 + j
                        nc.tensor.transpose(tp[:, j * 128:(j + 1) * 128],
                                            eq[:, p * 128:(p + 1) * 128], ident)
                    dst = qdst[:, 4 * g4:4 * g4 + 4, toff:toff + 128]
                    src = tp[:].rearrange("p (j t) -> p j t", j=4)
                    if g4 == 0:
                        nc.vector.tensor_copy(dst, src)
                    else:
                        nc.scalar.copy(dst, src)

            # ---- collective result -> normalized per-pair kv blocks ----
            kvred = accp.tile([128, NPAIR * 129], BF16, tag="kvred")
            nc.sync.dma_start(kvred, cout)
            rz = accp.tile([128, NPAIR], F32, tag="rz")
            nc.vector.reciprocal(
                rz, kvred[:].rearrange("q (p c) -> q p c", c=129)[:, :, 128])
            kv_sb = [kvsbp.tile([128, 128], BF16, tag=f"kv{p}", name=f"kv{p}")
                     for p in range(NPAIR)]
            for p in range(NPAIR):
                c0 = 129 * p
                # off-diagonal head-cross blocks must be exact zeros
                nc.vector.tensor_scalar_mul(
                    kv_sb[p][0:64, 64:128], kvred[0:64, c0:c0 + 64], 0.0)
                nc.vector.tensor_scalar_mul(
                    kv_sb[p][64:128, 0:64], kvred[64:128, c0:c0 + 64], 0.0)
                nc.vector.tensor_scalar_mul(
                    kv_sb[p][0:64, 0:64], kvred[0:64, c0:c0 + 64],
                    rz[0:64, p:p + 1])
                nc.vector.tensor_scalar_mul(
                    kv_sb[p][64:128, 64:128], kvred[64:128, c0 + 64:c0 + 128],
                    rz[64:128, p:p + 1])

            # ---- attention out + output projection, interleaved per
            # 512-token chunk; attn result overwrites the chunk's q tile
            for tcn in range(TCN):
                qtc = qt4[tcn]
                for p in range(NPAIR):
                    aps = psum.tile([128, 512], F32, tag="mm")
                    nc.tensor.matmul(aps, kv_sb[p], qtc[:, p, :],
                                     start=True, stop=True)
                    # split copybacks across ACT/DVE so neither engine gates
                    if p % 2 == 0:
                        nc.scalar.copy(qtc[:, p, :], aps)
                    else:
                        nc.vector.tensor_copy(qtc[:, p, :], aps)
                for t4 in range(4):
                    tb = 4 * tcn + t4
                    for oc in range(2):
                        osl = slice(oc * 512, (oc + 1) * 512)
                        ps = psum.tile([128, 512], F32, tag="mm")
                        for kc in range(KC):
                            nc.tensor.matmul(
                                ps, qtc[:, kc, t4 * 128:(t4 + 1) * 128],
                                wp_sb[kc][:, osl],
                                start=(kc == 0), stop=(kc == KC - 1))
                        yt = youtp.tile([128, 512], F32, tag="y")
                        if with_bias:
                            nc.vector.tensor_add(yt, ps, bp_sb[:, osl])
                        elif oc == 0:
                            nc.vector.tensor_copy(yt, ps)
                        else:
                            nc.scalar.copy(yt, ps)
                        nc.sync.dma_start(y[tb * 128:(tb + 1) * 128, osl], yt)

    nc.compile()
    return nc


_NC = {}


def _get_nc(with_bias=False):
    if with_bias not in _NC:
        _NC[with_bias] = build_program(with_bias=with_bias)
    return _NC[with_bias]


def kernel(x, w_qkv, b_qkv, w_proj, b_proj):
    x = np.asarray(x, dtype=np.float32)
    w_qkv = np.asarray(w_qkv, dtype=np.float32)
    b_qkv = np.asarray(b_qkv, dtype=np.float32)
    w_proj = np.asarray(w_proj, dtype=np.float32)
    b_proj = np.asarray(b_proj, dtype=np.float32)

    bs, seqlen, dim = x.shape
    half = seqlen // 2

    bf16 = ml_dtypes.bfloat16
    fp8 = ml_dtypes.float8_e4m3
    with_bias = bool(np.any(b_qkv)) or bool(np.any(b_proj))
    fp8_qk = FP8_QK and not with_bias

    wqT = np.ascontiguousarray(w_qkv[0:D].T)
    wkT = np.ascontiguousarray(w_qkv[D:2 * D].T)
    wvT = np.ascontiguousarray(w_qkv[2 * D:3 * D].T).astype(bf16)
    wpT = np.ascontiguousarray(w_proj.T).astype(bf16)
    bq, bk, bv = b_qkv[0:D], b_qkv[D:2 * D], b_qkv[2 * D:3 * D]

    cst = np.concatenate(
        [np.eye(128, dtype=np.float32),
         np.ones((128, 8), dtype=np.float32)], axis=1).astype(bf16)

    common = {
        "wv": wvT, "wp": wpT, "cst": cst,
        "bq": np.ascontiguousarray(bq), "bk": np.ascontiguousarray(bk),
        "bv": np.ascontiguousarray(bv), "bp": np.ascontiguousarray(b_proj),
    }
    if fp8_qk:
        common["wqf"] = (wqT * SW).astype(fp8)
        common["wkf"] = (wkT * SW).astype(fp8)
    else:
        common["wq"] = wqT.astype(bf16)
        common["wk"] = wkT.astype(bf16)

    in_maps = []
    for c in range(N_CORES):
        b, s = divmod(c, 2)
        chunk = np.ascontiguousarray(x[b, s * half:(s + 1) * half, :].T)
        m = dict(common)
        m["xt"] = chunk.astype(bf16)
        if fp8_qk:
            m["xf"] = (chunk * SX).astype(fp8)
        in_maps.append(m)

    nc = _get_nc(with_bias)
    global _last_in_maps, _last_with_bias
    _last_in_maps = in_maps
    _last_with_bias = with_bias
    res = bass_utils.run_bass_kernel_spmd(nc, in_maps, core_ids=list(range(N_CORES)))

    out = np.empty((bs, seqlen, dim), dtype=np.float32)
    for c in range(N_CORES):
        b, s = divmod(c, 2)
        out[b, s * half:(s + 1) * half, :] = res.results[c]["y"]
    return out


# revision 22
# speedup vs baseline: 1.2690x; 1.0166x over previous
"""Trainium2 Bass kernel for EfficientAttention (linear attention block).

Computation (per batch b, head h):
    qkv = x @ w_qkv.T (+ b_qkv)
    q = softmax(q, axis=head_dim) * head_dim**-0.5
    k = softmax(k, axis=seqlen)
    kv[d,e] = sum_s k[s,d] v[s,e]          (per-head 64x64 state)
    out[s,e] = sum_d q[s,d] kv[d,e]
    y = out @ w_proj.T (+ b_proj)

Sharding: 8 cores = (batch b = c//2, seq half = c%2); 2048 tokens per core,
all 16 heads. Cross-core coupling: the kv state and the k-softmax
denominator Z (sums over the full 4096 seqlen) -> one [128, 520] bf16
AllReduce across seq-half pairs.

Fast-path layout (no-bias variant, which is what zero-bias inputs use):
- q/k projections run in fp8(e4m3) DoubleRow mode (2 contraction chunks
  per instruction, 2x PE rate); inputs are host-scaled (x*16, w*512) to
  dodge fp8 subnormals and the exp() activation descales by 1/8192.
  v/out projections stay bf16 (their error hits the output linearly).
- All other operands bf16 on-chip; PSUM accumulation fp32.
- x resident in SBUF as per-token-block tiles (tile-granular dependency
  tracking makes a single big tile serialize on all its chunk DMAs).
- Z is fused into the kv-state matmul via a ones column appended to
  each v pair block.
- Program order: k/v sweep -> collective trigger -> whole q sweep
  (hides the AllReduce) -> attention + output projection interleaved
  per 512-token chunk (separate per-chunk q tiles avoid WAR stalls).
- DMA priority: wk + x block 0 first on the sync queue; wq/wp loads are
  gated behind tb0 so they don't steal startup HBM bandwidth.
"""

import sys

sys.path.insert(0, "/opt/trn_rl_repo")

import numpy as np
import ml_dtypes

import concourse.bacc as bacc
import concourse.tile as tile
from concourse import mybir
from concourse import bass_utils

F32 = mybir.dt.float32
BF16 = mybir.dt.bfloat16
FP8 = mybir.dt.float8e4

D = 1024          # model dim (= qkv contraction dim)
T = 2048          # tokens per core (one batch element's half sequence)
NH = 16           # heads
HD = 64           # head dim
NPAIR = 8         # head pairs (2 heads / 128 partitions)
KC = D // 128     # contraction chunks of 128
TB = T // 128     # token blocks of 128
SCALE = HD ** -0.5

SX = 16.0         # host scale on fp8 x
SW = 512.0        # host scale on fp8 wq/wk
DESCALE = 1.0 / (SX * SW)

N_CORES = 8
FP8_QK = True     # fp8 DoubleRow q/k projections in the no-bias program


def bias_bcast(b):
    # DRAM [D] broadcast-load to SBUF [128, D] (partition step 0)
    import concourse.bass as bass
    ap = b[:]
    return bass.AP(tensor=ap.tensor, offset=ap.offset, ap=[[0, 128]] + list(ap.ap))


def build_program(with_bias=False):
    fp8_qk = FP8_QK and not with_bias
    nc = bacc.Bacc("TRN2", target_bir_lowering=False, num_devices=N_CORES)

    xt = nc.dram_tensor("xt", [D, T], BF16, kind="ExternalInput")      # x.T
    wv = nc.dram_tensor("wv", [D, D], BF16, kind="ExternalInput")      # w_v.T
    wp = nc.dram_tensor("wp", [D, D], BF16, kind="ExternalInput")      # w_proj.T
    if fp8_qk:
        xf = nc.dram_tensor("xf", [D, T], FP8, kind="ExternalInput")   # x.T * SX
        wqf = nc.dram_tensor("wqf", [D, D], FP8, kind="ExternalInput")  # w_q.T * SW
        wkf = nc.dram_tensor("wkf", [D, D], FP8, kind="ExternalInput")  # w_k.T * SW
    else:
        wq = nc.dram_tensor("wq", [D, D], BF16, kind="ExternalInput")
        wk = nc.dram_tensor("wk", [D, D], BF16, kind="ExternalInput")
    bq = nc.dram_tensor("bq", [D], F32, kind="ExternalInput")
    bk = nc.dram_tensor("bk", [D], F32, kind="ExternalInput")
    bv = nc.dram_tensor("bv", [D], F32, kind="ExternalInput")
    bp = nc.dram_tensor("bp", [D], F32, kind="ExternalInput")
    cst = nc.dram_tensor("cst", [128, 136], BF16, kind="ExternalInput")  # identity | ones8
    y = nc.dram_tensor("y", [T, D], F32, kind="ExternalOutput")

    xt_v = xt.rearrange("(kc p) t -> p kc t", p=128)
    wv_v = wv.rearrange("(kc p) f -> p kc f", p=128)
    wp_v = wp.rearrange("(kc p) f -> p kc f", p=128)
    if fp8_qk:
        xf_v = xf.rearrange("(kc p) t -> p kc t", p=128)
        wqf_v = wqf.rearrange("(kc p) f -> p kc f", p=128)
        wkf_v = wkf.rearrange("(kc p) f -> p kc f", p=128)
    else:
        wq_v = wq.rearrange("(kc p) f -> p kc f", p=128)
        wk_v = wk.rearrange("(kc p) f -> p kc f", p=128)

    TCN = T // 512
    DR = mybir.MatmulPerfMode.DoubleRow

    with tile.TileContext(nc) as tc:
        with (
            tc.tile_pool(name="const", bufs=1) as const,
            tc.tile_pool(name="wpool", bufs=1) as wpool,
            tc.tile_pool(name="xp", bufs=1) as xp,
            tc.tile_pool(name="ekv", bufs=2) as ekv,
            tc.tile_pool(name="acc", bufs=1) as accp,
            tc.tile_pool(name="qpool", bufs=2) as qpool,
            tc.tile_pool(name="qt", bufs=1) as qtpool,
            tc.tile_pool(name="kvsb", bufs=1) as kvsbp,
            tc.tile_pool(name="yout", bufs=3) as youtp,
            tc.tile_pool(name="psum", bufs=3, space="PSUM") as psum,
            tc.tile_pool(name="dram", bufs=1, space="DRAM") as dram,
        ):
            cst_sb = const.tile([128, 136], BF16, tag="cst")
            nc.sync.dma_start(cst_sb, cst[:])
            ident = cst_sb[:, 0:128]

            # ---- weight tiles ----
            wv_sb = [wpool.tile([128, D], BF16, tag=f"wv{kc}", name=f"wv{kc}")
                     for kc in range(KC)]
            wp_sb = [wpool.tile([128, D], BF16, tag=f"wp{kc}", name=f"wp{kc}")
                     for kc in range(KC)]
            if fp8_qk:
                wkf_sb = wpool.tile([128, KC, D], FP8, tag="wkf", name="wkf")
                wqf_sb = wpool.tile([128, KC, D], FP8, tag="wqf", name="wqf")
                # sub-chunked so the pieces spread across DMA engines and
                # complete early (startup critical path)
                for kc in range(KC):
                    for hh in range(2):
                        nc.sync.dma_start(
                            wkf_sb[:, kc, 512 * hh:512 * (hh + 1)],
                            wkf_v[:, kc, 512 * hh:512 * (hh + 1)])
            else:
                wk_sb = [wpool.tile([128, D], BF16, tag=f"wk{kc}", name=f"wk{kc}")
                         for kc in range(KC)]
                wq_sb = [wpool.tile([128, D], BF16, tag=f"wq{kc}", name=f"wq{kc}")
                         for kc in range(KC)]
                for kc in range(KC):
                    nc.sync.dma_start(wk_sb[kc], wk_v[:, kc, :])

            if with_bias:
                bk_sb = const.tile([128, D], BF16, tag="bk")
                bv_sb = const.tile([128, D], BF16, tag="bv")
                bq_sb = const.tile([128, D], BF16, tag="bq")
                bp_sb = const.tile([128, D], BF16, tag="bp")
                nc.gpsimd.dma_start(bk_sb, bias_bcast(bk))
                nc.gpsimd.dma_start(bv_sb, bias_bcast(bv))
                nc.gpsimd.dma_start(bq_sb, bias_bcast(bq))
                nc.gpsimd.dma_start(bp_sb, bias_bcast(bp))

            # x resident in SBUF as per-tb tiles (tile-granular dep tracking:
            # one big tile would make tb0 wait for every chunk DMA)
            xts = [xp.tile([128, KC, 128], BF16, tag=f"xt{tb}", name=f"xt{tb}")
                   for tb in range(TB)]
            if fp8_qk:
                xfs = [xp.tile([128, KC, 128], FP8, tag=f"xf{tb}", name=f"xf{tb}")
                       for tb in range(TB)]

            def load_x(tb, eng):
                if fp8_qk:
                    eng.dma_start(xfs[tb], xf_v[:, :, tb * 128:(tb + 1) * 128])
                eng.dma_start(xts[tb], xt_v[:, :, tb * 128:(tb + 1) * 128])

            load_x(0, nc.sync)
            for kc in range(KC):
                nc.sync.dma_start(wv_sb[kc], wv_v[:, kc, :])
            for tb in range(1, TB):
                load_x(tb, nc.sync)

            # v tiles with a ones column per pair block: [v_pair(128) | 1] x 8
            vv_t = [accp.tile([128, NPAIR * 129], BF16, tag=f"v{i}", name=f"vv{i}")
                    for i in range(2)]
            for i in range(2):
                nc.vector.tensor_copy(
                    vv_t[i][:].rearrange("q (p c) -> q p c", c=129)[:, :, 128],
                    cst_sb[:, 128:136])

            def proj_qk(ps, tb, w_sb, sl):
                # q/k projection into psum: fp8 DoubleRow (2 kc per mm) or bf16
                if fp8_qk:
                    for g in range(KC // 2):
                        nc.tensor.matmul(ps, xfs[tb][:, 2 * g:2 * g + 2, :],
                                         w_sb[:, 2 * g:2 * g + 2, sl],
                                         start=(g == 0), stop=(g == KC // 2 - 1),
                                         perf_mode=DR)
                else:
                    for kc in range(KC):
                        nc.tensor.matmul(ps, xts[tb][:, kc, :], w_sb[kc][:, sl],
                                         start=(kc == 0), stop=(kc == KC - 1))

            exp_scale = DESCALE if fp8_qk else 1.0

            # ---- Phase B: k/v projections, exp(k), partial [kv | Z] ----
            kvacc = accp.tile([128, NPAIR * 129], F32, tag="kvacc")

            def phase_b_tb(tb):
                ek = ekv.tile([128, D], BF16, tag="ek", name="ek")
                vv = vv_t[tb % 2]
                for half in range(2):
                    sl = slice(half * 512, (half + 1) * 512)
                    ps = psum.tile([128, 512], F32, tag="mm")
                    proj_qk(ps, tb, wkf_sb if fp8_qk else wk_sb, sl)
                    if with_bias:
                        nc.vector.tensor_add(ps, ps, bk_sb[:, sl])
                    nc.scalar.activation(ek[:, sl], ps,
                                         mybir.ActivationFunctionType.Exp,
                                         scale=exp_scale)
                    ps = psum.tile([128, 512], F32, tag="mm")
                    for kc in range(KC):
                        nc.tensor.matmul(ps, xts[tb][:, kc, :], wv_sb[kc][:, sl],
                                         start=(kc == 0), stop=(kc == KC - 1))
                    if with_bias:
                        nc.vector.tensor_add(ps, ps, bv_sb[:, sl])
                    for j in range(4):
                        p = 4 * half + j
                        dst = vv[:, 129 * p:129 * p + 128]
                        src = ps[:, 128 * j:128 * j + 128]
                        if j % 2 == 0:
                            nc.scalar.copy(dst, src)
                        else:
                            nc.vector.tensor_copy(dst, src)
                # [kv | Z] partials: per pair, lhsT = ek pair cols,
                # rhs = [v pair cols | ones] (129 moving)
                for g in range(4):
                    kps = psum.tile([128, 258], F32, tag="kv", bufs=2)
                    for j in range(2):
                        p = 2 * g + j
                        nc.tensor.matmul(
                            kps[:, 129 * j:129 * j + 129],
                            ek[:, 128 * p:128 * p + 128],
                            vv[:, 129 * p:129 * p + 129],
                            start=True, stop=True)
                    if tb == 0:
                        nc.vector.tensor_copy(kvacc[:, 258 * g:258 * (g + 1)], kps)
                    else:
                        nc.vector.tensor_add(kvacc[:, 258 * g:258 * (g + 1)],
                                             kvacc[:, 258 * g:258 * (g + 1)], kps)
                return ek

            ek0 = phase_b_tb(0)
            # release wq/wp weight loads only once tb0 is in flight: a tiny
            # gpsimd DMA depending on ek0 stalls the gpsimd queue (in-order)
            # so the big loads don't compete with wk/wv/x for HBM at startup
            gate = dram.tile([1, 8], BF16, tag="gate")
            nc.gpsimd.dma_start(gate, ek0[0:1, 0:8])
            if fp8_qk:
                for kc in range(KC):
                    nc.gpsimd.dma_start(wqf_sb[:, kc, :], wqf_v[:, kc, :])
            else:
                for kc in range(KC):
                    nc.gpsimd.dma_start(wq_sb[kc], wq_v[:, kc, :])
            for kc in range(KC):
                nc.gpsimd.dma_start(wp_sb[kc], wp_v[:, kc, :])
            for tb in range(1, TB):
                phase_b_tb(tb)

            # ---- stage = whole [kv | Z] pair-block accumulator cast to bf16
            # in ONE op (keeps the DVE queue clear for the q sweep); the
            # AllReduce carries the cross-head garbage blocks too (264 KB),
            # which stays hidden under the q sweep
            stage = accp.tile([128, NPAIR * 129], BF16, tag="stage")
            nc.vector.tensor_copy(stage, kvacc)
            cin = dram.tile([128, NPAIR * 129], BF16, tag="cin")
            cout = dram.tile([128, NPAIR * 129], BF16, tag="cout")
            nc.sync.dma_start(cin, stage)
            nc.gpsimd.collective_compute(
                "AllReduce", mybir.AluOpType.add,
                replica_groups=[[0, 1], [2, 3], [4, 5], [6, 7]],
                ins=[cin[:].opt()], outs=[cout[:].opt()])

            # ---- q sweep (independent of the collective; hides it) ----
            # per-512-token-chunk q tiles so later attn writes don't WAR-stall
            qt4 = [qtpool.tile([128, NPAIR, 512], BF16, tag=f"qt{i}", name=f"qt{i}")
                   for i in range(TCN)]
            import concourse.bass as bass
            for tb in range(TB):
                eq = qpool.tile([128, D], BF16, tag="eq", bufs=3)
                sums = qpool.tile([128, NH], F32, tag="sums", bufs=3)
                for half in range(2):
                    sl = slice(half * 512, (half + 1) * 512)
                    ps = psum.tile([128, 512], F32, tag="mm")
                    proj_qk(ps, tb, wqf_sb if fp8_qk else wq_sb, sl)
                    if with_bias:
                        nc.vector.tensor_add(ps, ps, bq_sb[:, sl])
                    nc.scalar.activation(eq[:, sl], ps,
                                         mybir.ActivationFunctionType.Exp,
                                         scale=exp_scale)
                    # half-granular row sums pipeline behind the exps
                    nc.vector.reduce_sum(
                        sums[:, 8 * half:8 * half + 8],
                        eq[:, sl].rearrange("p (h e) -> p h e", e=HD),
                        axis=mybir.AxisListType.X)
                rfacf = qpool.tile([128, NH], F32, tag="rfacf", bufs=3)
                nc.vector.reciprocal(rfacf, sums)
                rfac = qpool.tile([128, NH], BF16, tag="rfac", bufs=3)
                nc.scalar.mul(rfac, rfacf, SCALE)
                # normalize all 16 heads in one DVE op: rfac broadcast along
                # head_dim via a stride-0 inner AP (all-bf16 for 2x DVE rate)
                rap = rfac[:, :]
                rbc = bass.AP(tensor=rap.tensor, offset=rap.offset,
                              ap=[list(rap.ap[0]), list(rap.ap[1]), [0, HD]])
                eqv = eq[:].rearrange("p (h e) -> p h e", e=HD)
                nc.vector.tensor_mul(eqv, eqv, rbc)
                qdst = qt4[tb // 4]
                toff = (tb % 4) * 128
                for g4 in range(2):
                    tp = psum.tile([128, 512], BF16, tag="tr", bufs=3)
                    for j in range(4):
                        p = 4 * g4 + j
                        nc.tensor.transpose(tp[:, j * 128:(j + 1) * 128],
                                            eq[:, p * 128:(p + 1) * 128], ident)
                    dst = qdst[:, 4 * g4:4 * g4 + 4, toff:toff + 128]
                    src = tp[:].rearrange("p (j t) -> p j t", j=4)
                    # both copies on ACT: DVE is the q-sweep's tightest engine
                    nc.scalar.copy(dst, src)

            # ---- collective result -> normalized per-pair kv blocks ----
            kvred = accp.tile([128, NPAIR * 129], BF16, tag="kvred")
            nc.sync.dma_start(kvred, cout)
            rz = accp.tile([128, NPAIR], F32, tag="rz")
            nc.vector.reciprocal(
                rz, kvred[:].rearrange("q (p c) -> q p c", c=129)[:, :, 128])
            kv_sb = [kvsbp.tile([128, 128], BF16, tag=f"kv{p}", name=f"kv{p}")
                     for p in range(NPAIR)]
            for p in range(NPAIR):
                c0 = 129 * p
                # off-diagonal head-cross blocks must be exact zeros
                nc.vector.tensor_scalar_mul(
                    kv_sb[p][0:64, 64:128], kvred[0:64, c0:c0 + 64], 0.0)
                nc.vector.tensor_scalar_mul(
                    kv_sb[p][64:128, 0:64], kvred[64:128, c0:c0 + 64], 0.0)
                nc.vector.tensor_scalar_mul(
                    kv_sb[p][0:64, 0:64], kvred[0:64, c0:c0 + 64],
                    rz[0:64, p:p + 1])
                nc.vector.tensor_scalar_mul(
                    kv_sb[p][64:128, 64:128], kvred[64:128, c0 + 64:c0 + 128],
                    rz[64:128, p:p + 1])

            # ---- attention out + output projection, interleaved per
            # 512-token chunk; attn result overwrites the chunk's q tile
            for tcn in range(TCN):
                qtc = qt4[tcn]
                for p in range(NPAIR):
                    aps = psum.tile([128, 512], F32, tag="mm")
                    nc.tensor.matmul(aps, kv_sb[p], qtc[:, p, :],
                                     start=True, stop=True)
                    # split copybacks across ACT/DVE so neither engine gates
                    if p % 2 == 0:
                        nc.scalar.copy(qtc[:, p, :], aps)
                    else:
                        nc.vector.tensor_copy(qtc[:, p, :], aps)
                for t4 in range(4):
                    tb = 4 * tcn + t4
                    for oc in range(2):
                        osl = slice(oc * 512, (oc + 1) * 512)
                        ps = psum.tile([128, 512], F32, tag="mm")
                        for kc in range(KC):
                            nc.tensor.matmul(
                                ps, qtc[:, kc, t4 * 128:(t4 + 1) * 128],
                                wp_sb[kc][:, osl],
                                start=(kc == 0), stop=(kc == KC - 1))
                        yt = youtp.tile([128, 512], F32, tag="y")
                        if with_bias:
                            nc.vector.tensor_add(yt, ps, bp_sb[:, osl])
                        elif oc == 0:
                            nc.vector.tensor_copy(yt, ps)
                        else:
                            nc.scalar.copy(yt, ps)
                        nc.sync.dma_start(y[tb * 128:(tb + 1) * 128, osl], yt)

    nc.compile()
    return nc


_NC = {}


def _get_nc(with_bias=False):
    if with_bias not in _NC:
        _NC[with_bias] = build_program(with_bias=with_bias)
    return _NC[with_bias]


def kernel(x, w_qkv, b_qkv, w_proj, b_proj):
    x = np.asarray(x, dtype=np.float32)
    w_qkv = np.asarray(w_qkv, dtype=np.float32)
    b_qkv = np.asarray(b_qkv, dtype=np.float32)
    w_proj = np.asarray(w_proj, dtype=np.float32)
    b_proj = np.asarray(b_proj, dtype=np.float32)

    bs, seqlen, dim = x.shape
    half = seqlen // 2

    bf16 = ml_dtypes.bfloat16
    fp8 = ml_dtypes.float8_e4m3
    with_bias = bool(np.any(b_qkv)) or bool(np.any(b_proj))
    fp8_qk = FP8_QK and not with_bias

    wqT = np.ascontiguousarray(w_qkv[0:D].T)
    wkT = np.ascontiguousarray(w_qkv[D:2 * D].T)
    wvT = np.ascontiguousarray(w_qkv[2 * D:3 * D].T).astype(bf16)
    wpT = np.ascontiguousarray(w_proj.T).astype(bf16)
    bq, bk, bv = b_qkv[0:D], b_qkv[D:2 * D], b_qkv[2 * D:3 * D]

    cst = np.concatenate(
        [np.eye(128, dtype=np.float32),
         np.ones((128, 8), dtype=np.float32)], axis=1).astype(bf16)

    common = {
        "wv": wvT, "wp": wpT, "cst": cst,
        "bq": np.ascontiguousarray(bq), "bk": np.ascontiguousarray(bk),
        "bv": np.ascontiguousarray(bv), "bp": np.ascontiguousarray(b_proj),
    }
    if fp8_qk:
        common["wqf"] = (wqT * SW).astype(fp8)
        common["wkf"] = (wkT * SW).astype(fp8)
    else:
        common["wq"] = wqT.astype(bf16)
        common["wk"] = wkT.astype(bf16)

    in_maps = []
    for c in range(N_CORES):
        b, s = divmod(c, 2)
        chunk = np.ascontiguousarray(x[b, s * half:(s + 1) * half, :].T)
        m = dict(common)
        m["xt"] = chunk.astype(bf16)
        if fp8_qk:
            m["xf"] = (chunk * SX).astype(fp8)
        in_maps.append(m)

    nc = _get_nc(with_bias)
    global _last_in_maps, _last_with_bias
    _last_in_maps = in_maps
    _last_with_bias = with_bias
    res = bass_utils.run_bass_kernel_spmd(nc, in_maps, core_ids=list(range(N_CORES)))

    out = np.empty((bs, seqlen, dim), dtype=np.float32)
    for c in range(N_CORES):
        b, s = divmod(c, 2)
        out[b, s * half:(s + 1) * half, :] = res.results[c]["y"]
    return out
